# revision 46
# baseline (speedup 1.0000x reference)
"""GQA decode attention (B=16, S=4096, NH=32, NKV=8, HD=128) on 8 TRN2 cores.

Sharding: tensor-parallel over heads — 1 KV head (4 Q heads) per core.
Each core: qkv projection for its 768 wqkv rows, RoPE + QK-RMSNorm,
attention over its KV-head slice of the caches, RowParallel o_proj slice
producing a [16, 4096] partial; partials are summed on the host.

v2 (fp16 streaming): K cache, V cache, wqkv and o_proj weights are all
streamed in fp16 (harness gate is rel_err < 2e-2; fp16 end-to-end error is
~1e-3). This halves HBM traffic vs the bf16-hi/lo baseline and runs all
large matmuls with 1-cycle/col moving operands.

The cache scatter at last_pos (host-known, baked in at compile time) is
done exactly, with no correction matmuls:
 - after exp, e_new = exp(q.k_new) overwrites the stale column's weights
   (expt[last_pos%128, 4*chunk:4*chunk+4] = e_new), and
 - v_new overwrites the stale V row in the streamed V tile.
Then numerator (V matmuls) and softmax denominator (ones-vector matmul
over expt) are both exact. Softmax skips max-subtraction (scores ~N(0,1)
after QK-RMSNorm, exp fits fp16); normalization is one reciprocal-multiply
on the accumulated [d, h, b] output before o_proj.
"""

import sys
from contextlib import ExitStack

for _p in ("/opt/trn_rl_repo",):
    if _p not in sys.path:
        sys.path.insert(0, _p)

import numpy as np

import concourse.bass as bass
import concourse.tile as tile
from concourse import mybir
from concourse.bass_utils import run_bass_kernel_spmd
from concourse.masks import make_identity

B, S, H = 16, 4096, 4096
NH, NKV, HD = 32, 8, 128
NREP = NH // NKV  # 4 q heads per kv head (= per core)
DQ = NREP * HD  # 512
NCORES = 8
EPS = 1e-5
NCH = S // 128  # 32 seq chunks
F32 = mybir.dt.float32
F16 = mybir.dt.float16
AF = mybir.ActivationFunctionType
AX = mybir.AxisListType


def _legalize_waits(nc):
    """This walrus build accepts at most ONE sync wait on most instruction
    encodings (Matmult's S3_LW, DMA structs, ...) while Tile may attach
    several. Move excess waits onto same-engine no-ops inserted right before
    the instruction (semantically identical: the engine queue executes the
    wait no-ops, then the instruction)."""
    moved = 0
    skip = (mybir.InstNoOp, mybir.InstEventSemaphore)
    for func in nc.m.functions:
        for bb in func.blocks:
            insts = list(bb.instructions)
            out = []
            changed = False
            for inst in insts:
                si = inst.sync_info
                if (
                    si is not None
                    and si.on_wait
                    and len(si.on_wait) > 1
                    and not isinstance(inst, skip)
                ):
                    waits = list(si.on_wait)
                    for k, w in enumerate(waits[:-1]):
                        nop = mybir.InstNoOp(
                            name=f"{inst.name}-w{k}", engine=inst.engine
                        )
                        nop.sync_info = mybir.SyncInfo(on_wait=[w], on_update=[])
                        out.append(nop)
                        moved += 1
                    si.on_wait = waits[-1:]
                    inst.sync_info = si
                    changed = True
                out.append(inst)
            if changed:
                bb.instructions = out
    return moved


def _build_bass(lp, legalize=True, reps=1, kvbufs=4, exp_f16=True):
    """Build the SPMD Bass program. lp: tuple of 16 ints (last_pos, baked).

    reps > 1 repeats the whole computation (for slope-based timing: the
    per-call dispatch overhead cancels between two rep counts)."""
    nc = bass.Bass("TRN2", target_bir_lowering=False, debug=False)

    xt_d = nc.dram_tensor("xt", [128, NCH, B], F16, kind="ExternalInput")
    wq_d = nc.dram_tensor("wq", [NCH, 128, 768], F16, kind="ExternalInput")
    kt_d = nc.dram_tensor("kt", [B, 128, S], F16, kind="ExternalInput")
    vv_d = nc.dram_tensor("vv", [B, 128, NCH, HD], F16, kind="ExternalInput")
    ow_d = nc.dram_tensor("ow", [NREP, 128, 8, 512], F16, kind="ExternalInput")
    sel_d = nc.dram_tensor("selmat", [B, 128], F16, kind="ExternalInput")
    cosq_d = nc.dram_tensor("cosq", [B, NREP, 64], F32, kind="ExternalInput")
    sinq_d = nc.dram_tensor("sinq", [B, NREP, 64], F32, kind="ExternalInput")
    cosk_d = nc.dram_tensor("cosk", [B, 64], F32, kind="ExternalInput")
    sink_d = nc.dram_tensor("sink", [B, 64], F32, kind="ExternalInput")
    out_d = nc.dram_tensor("out_p", [B, H], F32, kind="ExternalOutput")

    with tile.TileContext(nc) as tc, ExitStack() as ctx:
        consts = ctx.enter_context(tc.tile_pool(name="consts", bufs=1))
        sb = ctx.enter_context(tc.tile_pool(name="sb", bufs=2))
        kpool = ctx.enter_context(tc.tile_pool(name="kpool", bufs=kvbufs))
        vpool = ctx.enter_context(tc.tile_pool(name="vpool", bufs=kvbufs))
        wpool = ctx.enter_context(tc.tile_pool(name="wpool", bufs=3))
        epool = ctx.enter_context(tc.tile_pool(name="epool", bufs=2))

        ident = consts.tile([128, 128], F32)
        make_identity(nc, ident[:, :])

        xt_sb = consts.tile([128, NCH, B], F16)
        nc.sync.dma_start(out=xt_sb[:, :, :], in_=xt_d[:, :, :])
        cosq = consts.tile([B, NREP, 64], F32)
        sinq = consts.tile([B, NREP, 64], F32)
        cosk = consts.tile([B, 64], F32)
        sink = consts.tile([B, 64], F32)
        epsq = consts.tile([B, 1], F32)
        epsk = consts.tile([B, 1], F32)
        nc.vector.memset(epsq[:, :], float(HD * EPS))
        nc.vector.memset(epsk[:, :], float(EPS))
        nc.sync.dma_start(out=cosq[:, :, :], in_=cosq_d[:, :, :])
        nc.sync.dma_start(out=sinq[:, :, :], in_=sinq_d[:, :, :])
        nc.sync.dma_start(out=cosk[:, :], in_=cosk_d[:, :])
        nc.sync.dma_start(out=sink[:, :], in_=sink_d[:, :])
        ones128 = consts.tile([128, 1], F16)
        nc.vector.memset(ones128[:, :], 1.0)
        selmat = consts.tile([B, 128], F16)
        nc.sync.dma_start(out=selmat[:, :], in_=sel_d[:, :])
        onesrow = consts.tile([1, 128], F32)
        nc.vector.memset(onesrow[:, :], 1.0)

        for rep in range(reps):
            qn = consts.tile([B, NREP, 64, 2], F32)  # rope'd+normed q (with 1/sqrt(HD))
            kn = consts.tile([B, HD], F32)  # rope'd+normed k
            vn = consts.tile([B, HD], F16)  # new v row (fp16)
            enew = consts.tile([B, NREP], F32)  # exp(q . k_new)
            qT16 = consts.tile([128, B * NREP], F16)  # col b*4+h
            oT_sb = consts.tile([128, NREP, B], F32)  # attention out, [d, (g, b)]
            oT16 = consts.tile([128, NREP, B], F16)  # normalized, fp16
            den_sb = consts.tile([1, B, NREP], F32)  # softmax denominators

            # ---- qkv projection: qkv[b, o] = sum_h x[b, h] * wqkv_c[o, h] ----
            with tc.tile_pool(name="psq", bufs=1, space="PSUM") as psq:
                ps_qkv = psq.tile([B, 768], F32)
                for ii in range(NCH // 4):
                    wt = wpool.tile([128, 4, 768], F16, tag="wq")
                    nc.gpsimd.dma_start(
                        out=wt[:, :, :],
                        in_=wq_d[4 * ii:4 * ii + 4, :, :].transpose([1, 0, 2]),
                    )
                    for k in range(4):
                        i = 4 * ii + k
                        nc.tensor.matmul(
                            ps_qkv[:, 0:512], xt_sb[:, i, :], wt[:, k, 0:512],
                            start=(i == 0), stop=(i == NCH - 1),
                        )
                        nc.tensor.matmul(
                            ps_qkv[:, 512:768], xt_sb[:, i, :], wt[:, k, 512:768],
                            start=(i == 0), stop=(i == NCH - 1),
                        )
                ps_q = ps_qkv[:, 0:DQ].rearrange("p (a b c) -> p a b c", b=64, c=2)
                q_ev, q_od = ps_q[:, :, :, 0], ps_q[:, :, :, 1]
                ps_k = ps_qkv[:, DQ:DQ + HD].rearrange("p (b c) -> p b c", c=2)
                k_ev, k_od = ps_k[:, :, 0], ps_k[:, :, 1]
                v_new = ps_qkv[:, DQ + HD:768]

                # ---- RoPE (interleaved pairs) + QK-RMSNorm, in [B, .] layout ----
                t0 = sb.tile([B, NREP, 64], F32, tag="t0")
                t1 = sb.tile([B, NREP, 64], F32, tag="t1")
                nc.vector.tensor_mul(t0[:, :, :], q_ev, cosq[:, :, :])
                nc.vector.tensor_mul(t1[:, :, :], q_od, sinq[:, :, :])
                nc.vector.tensor_sub(qn[:, :, :, 0], t0[:, :, :], t1[:, :, :])
                nc.vector.tensor_mul(t0[:, :, :], q_od, cosq[:, :, :])
                nc.vector.tensor_mul(t1[:, :, :], q_ev, sinq[:, :, :])
                nc.vector.tensor_add(qn[:, :, :, 1], t0[:, :, :], t1[:, :, :])

                kn2 = kn[:, :].rearrange("p (a b) -> p a b", b=2)
                t2 = sb.tile([B, 64], F32, tag="t2")
                t3 = sb.tile([B, 64], F32, tag="t3")
                nc.vector.tensor_mul(t2[:, :], k_ev, cosk[:, :])
                nc.vector.tensor_mul(t3[:, :], k_od, sink[:, :])
                nc.vector.tensor_sub(kn2[:, :, 0], t2[:, :], t3[:, :])
                nc.vector.tensor_mul(t2[:, :], k_od, cosk[:, :])
                nc.vector.tensor_mul(t3[:, :], k_ev, sink[:, :])
                nc.vector.tensor_add(kn2[:, :, 1], t2[:, :], t3[:, :])

                # new v row (fp16, no rope/norm)
                nc.vector.tensor_copy(vn[:, :], v_new)

            # ---- o_proj weights: prefetch all 4 head-groups now (gpsimd queue,
            # behind the wq chunks; consumed only at the end, overlaps attention).
            owts = []
            for g in range(NREP):
                owt = consts.tile([128, 8, 512], F16, tag=f"ow{g}")
                nc.gpsimd.dma_start(out=owt[:, :, :], in_=ow_d[g, :, :, :])
                owts.append(owt)

            # RMSNorm q; fold in the 1/sqrt(HD) score scale:
            # rstd' = 1/sqrt(ssq + HD*eps) = rsqrt(mean(q^2)+eps)/sqrt(HD)
            qn128 = qn[:, :, :, :].rearrange("p a b c -> p a (b c)")  # [16, 4, 128]
            sq = sb.tile([B, NREP, HD], F32, tag="sq")
            nc.vector.tensor_mul(sq[:, :, :], qn128, qn128)
            ssq = sb.tile([B, NREP, 1], F32, tag="ssq")
            nc.vector.reduce_sum(out=ssq[:, :, :], in_=sq[:, :, :], axis=AX.X)
            rstdq = sb.tile([B, NREP, 1], F32, tag="rstdq")
            nc.scalar.activation(rstdq[:, :, :], ssq[:, :, :], AF.Sqrt, bias=epsq[:, :])
            nc.vector.reciprocal(rstdq[:, :, :], rstdq[:, :, :])
            for h in range(NREP):
                nc.vector.tensor_scalar_mul(qn128[:, h, :], qn128[:, h, :], rstdq[:, h, :])

            # RMSNorm k (no extra scale)
            sk = sb.tile([B, HD], F32, tag="sk")
            nc.vector.tensor_mul(sk[:, :], kn[:, :], kn[:, :])
            ssk = sb.tile([B, 1], F32, tag="ssk")
            nc.vector.reduce_sum(out=ssk[:, :], in_=sk[:, :], axis=AX.X)
            nc.scalar.activation(ssk[:, :], ssk[:, :], AF.Sqrt, scale=1.0 / HD, bias=epsk[:, :])
            nc.vector.reciprocal(ssk[:, :], ssk[:, :])
            nc.vector.tensor_scalar_mul(kn[:, :], kn[:, :], ssk[:, :])

            # s_new[b, h] = qn . kn (scale already folded into qn); e_new = exp
            prod = sb.tile([B, NREP, HD], F32, tag="prod")
            kb = kn[:, :].unsqueeze(1).broadcast_to((B, NREP, HD))
            nc.vector.tensor_mul(prod[:, :, :], qn128, kb)
            snew = consts.tile([B, NREP, 1], F32)
            nc.vector.reduce_sum(out=snew[:, :, :], in_=prod[:, :, :], axis=AX.X)
            nc.scalar.activation(enew[:, :].unsqueeze(2), snew[:, :, :], AF.Exp)

            # ---- transpose q to [HD, .] layout via PE, cast to fp16 ----
            with tc.tile_pool(name="psT", bufs=1, space="PSUM") as psT:
                ps_qT = psT.tile([128, NREP * B], F32)  # col h*16+b
                for h in range(NREP):
                    nc.tensor.transpose(
                        ps_qT[:, h * B:(h + 1) * B],
                        qn128[:, h, :],
                        ident[0:B, 0:B],
                    )
                # reorder h*16+b -> b*4+h while copying to SBUF (f32 -> f16)
                qT_src = ps_qT[:, :].rearrange("p (h b) -> p b h", h=NREP)
                qT_dst = qT16[:, :].rearrange("p (b h) -> p b h", h=NREP)
                nc.vector.tensor_copy(qT_dst, qT_src)

            # ---- attention over the streamed caches ----
            with (
                tc.tile_pool(name="psc", bufs=2, space="PSUM") as psc_pool,
                tc.tile_pool(name="pso", bufs=1, space="PSUM") as pso_pool,
                tc.tile_pool(name="psd", bufs=2, space="PSUM") as psd_pool,
                tc.tile_pool(name="psb", bufs=1, space="PSUM") as psb_pool,
                tc.tile_pool(name="psO", bufs=2, space="PSUM") as psO,
            ):
                kts, vvs, pscs = {}, {}, {}

                def issue_dma(b):
                    # queue rebalance: the last two batches' K/V ride the
                    # gpsimd (SWDGE) queue behind wq+ow, evening the three
                    # DMA queues at ~14.7 MB each in case throughput is
                    # per-queue-limited rather than aggregate-limited.
                    keng = nc.gpsimd if b >= B - 2 else nc.sync
                    veng = nc.gpsimd if b >= B - 2 else nc.scalar
                    kts[b] = kpool.tile([128, S], F16, tag="kt", name=f"kt{b}")
                    keng.dma_start(out=kts[b][:, :], in_=kt_d[b, :, :])
                    vvs[b] = vpool.tile(
                        [128, NCH, HD], F16, tag="vv", name=f"vv{b}"
                    )
                    veng.dma_start(out=vvs[b][:, :, :], in_=vv_d[b, :, :, :])

                def scores(b):
                    jb = lp[b] // 128
                    # mask s_new down to batch b's row (partition b)
                    snew_b = sb.tile([B, NREP], F16, tag="snewb")
                    nc.vector.tensor_scalar_mul(
                        snew_b[:, :],
                        snew[:, :, :].rearrange("p a b -> p (a b)"),
                        ident[0:B, b:b + 1],
                    )
                    psc = pscs[b] = psc_pool.tile(
                        [128, 128], F32, tag="psc", name=f"psc{b}"
                    )
                    for j in range(NCH):
                        nc.tensor.matmul(
                            psc[:, 4 * j:4 * j + 4],
                            kts[b][:, 128 * j:128 * (j + 1)],
                            qT16[:, 4 * b:4 * b + 4],
                            start=True, stop=(j != jb),
                        )
                        if j == jb:
                            # scatter: the stale K column was zeroed on the
                            # host, so adding selmat^T @ snew_b puts s_new on
                            # partition last_pos%128 — exp then yields e_new
                            # there, making denominator and weights exact.
                            nc.tensor.matmul(
                                psc[:, 4 * j:4 * j + 4],
                                selmat[:, :], snew_b[:, :],
                                start=False, stop=True,
                            )

                HB = B // 2

                def emit_half(half):
                    # normalize (broadcast 1/den via PE) + o_proj for one
                    # b-half; half 0 is emitted mid-loop so it overlaps the
                    # second half's attention streaming.
                    b0 = half * HB
                    ps_bc = psb_pool.tile(
                        [128, HB * NREP], F32, tag="psbc", name=f"psbc{half}"
                    )
                    den_flat = den_sb[:, :, :].rearrange("p b h -> p (b h)")
                    nc.tensor.matmul(
                        ps_bc[:, :], onesrow[:, :],
                        den_flat[:, b0 * NREP:(b0 + HB) * NREP],
                        start=True, stop=True,
                    )
                    rec = sb.tile([128, HB * NREP], F32, tag="rec")
                    nc.vector.reciprocal(rec[:, :], ps_bc[:, :])
                    rec_v = rec[:, :].rearrange("p (b h) -> p h b", h=NREP)
                    nc.vector.tensor_mul(
                        oT16[:, :, b0:b0 + HB], oT_sb[:, :, b0:b0 + HB], rec_v
                    )
                    for nb in range(8):
                        ps_out = psO.tile(
                            [HB, 512], F32, tag="po", name=f"po{half}_{nb}"
                        )
                        for g in range(NREP):
                            nc.tensor.matmul(
                                ps_out[:, :], oT16[:, g, b0:b0 + HB],
                                owts[g][:, nb, :],
                                start=(g == 0), stop=(g == NREP - 1),
                            )
                        out_sb = sb.tile([HB, 512], F32, tag="outsb")
                        nc.vector.tensor_copy(out_sb[:, :], ps_out[:, :])
                        nc.sync.dma_start(
                            out=out_d[b0:b0 + HB, 512 * nb:512 * (nb + 1)],
                            in_=out_sb[:, :],
                        )

                # prologue: prefetch, then pipeline so scores(b+1) fills the
                # PE while the exp activation for b completes
                for b in range(3):
                    issue_dma(b)
                scores(0)
                for b in range(B):
                    if b + 3 < B:
                        issue_dma(b + 3)
                    expt = epool.tile([128, 128], F16 if exp_f16 else F32, tag="expt")
                    nc.scalar.activation(expt[:, :], pscs[b][:, :], AF.Exp)
                    if b + 1 < B:
                        scores(b + 1)

                    enew_b = sb.tile([B, NREP], F16, tag="enewb")
                    nc.vector.tensor_scalar_mul(
                        enew_b[:, :], enew[:, :], ident[0:B, b:b + 1]
                    )
                    # numerator: oT[d, h] += sum_s v[s, d] * e[s, h]; the stale
                    # V row was zeroed on the host, its true contribution
                    # e_new * v_new is the rank-1 matmul at the end.
                    psoT = pso_pool.tile([128, NREP], F32, tag="psoT")
                    for j in range(NCH):
                        nc.tensor.matmul(
                            psoT[:, :], vvs[b][:, j, :], expt[:, 4 * j:4 * j + 4],
                            start=(j == 0), stop=False,
                        )
                    nc.tensor.matmul(
                        psoT[:, :], vn[:, :], enew_b[:, :],
                        start=False, stop=True,
                    )
                    nc.vector.tensor_copy(oT_sb[:, :, b], psoT[:, :])

                    # denominator: one ones-vector matmul + reduce over chunks
                    psd = psd_pool.tile([1, 128], F32, tag="psd")
                    nc.tensor.matmul(
                        psd[:, :], ones128[:, :], expt[:, :],
                        start=True, stop=True,
                    )
                    den_v = psd[:, :].rearrange("p (j h) -> p h j", h=NREP)
                    nc.vector.reduce_sum(
                        out=den_sb[0:1, b, :].unsqueeze(2), in_=den_v, axis=AX.X
                    )

                    if b == HB - 1:
                        emit_half(0)
                emit_half(1)

    if legalize:
        _legalize_waits(nc)
    return nc


def _prep_inputs(x, last_pos, rope_cache, wqkv, o_proj_w, cache_k, cache_v):
    f32, f16 = np.float32, np.float16
    x2 = np.asarray(x, f32).reshape(B, H)
    lp = tuple(int(v) for v in np.asarray(last_pos).reshape(-1))
    rc = np.asarray(rope_cache, f32)[list(lp)]  # [16, 64, 2]
    cos, sin = rc[..., 0].copy(), rc[..., 1].copy()  # [16, 64]
    cosq = np.ascontiguousarray(np.broadcast_to(cos[:, None, :], (B, NREP, 64)))
    sinq = np.ascontiguousarray(np.broadcast_to(sin[:, None, :], (B, NREP, 64)))

    xt = np.ascontiguousarray(
        x2.T.reshape(NCH, 128, B).transpose(1, 0, 2)
    ).astype(f16)

    wqkv = np.asarray(wqkv, f32)
    o_proj_w = np.asarray(o_proj_w, f32)
    cache_k = np.asarray(cache_k)
    cache_v = np.asarray(cache_v)

    # [8, 16, 128, 4096] : per-core K^T, fp16
    ktall = np.ascontiguousarray(cache_k.transpose(2, 0, 3, 1)).astype(f16)
    # [8, 16, 128, 32, 128] : per-core V, chunk-major repack, fp16
    v5 = cache_v.reshape(B, NCH, 128, NKV, HD).transpose(3, 0, 2, 1, 4)
    vvall = np.ascontiguousarray(v5).astype(f16)
    # zero the stale K column / V row at last_pos; their true (new-token)
    # contribution is re-added exactly on-device via the scatter matmuls
    selmat = np.zeros((B, 128), f16)
    for b in range(B):
        ktall[:, b, :, lp[b]] = 0
        vvall[:, b, lp[b] % 128, lp[b] // 128, :] = 0
        selmat[b, lp[b] % 128] = 1.0

    per_core = []
    for c in range(NCORES):
        w_c = np.concatenate(
            [
                wqkv[c * DQ:(c + 1) * DQ],
                wqkv[NH * HD + c * HD:NH * HD + (c + 1) * HD],
                wqkv[NH * HD + NKV * HD + c * HD:NH * HD + NKV * HD + (c + 1) * HD],
            ],
            axis=0,
        )  # [768, 4096]
        wq_c = np.ascontiguousarray(w_c.T).reshape(NCH, 128, 768).astype(f16)
        ow_c = (
            np.ascontiguousarray(o_proj_w[:, c * DQ:(c + 1) * DQ].T)
            .reshape(NREP, 128, 8, 512)
            .astype(f16)
        )
        per_core.append(
            {
                "xt": xt,
                "wq": wq_c,
                "kt": ktall[c],
                "vv": vvall[c],
                "ow": ow_c,
                "selmat": selmat,
                "cosq": cosq,
                "sinq": sinq,
                "cosk": cos,
                "sink": sin,
            }
        )
    return lp, per_core


_NC_CACHE = {}
LAST_RESULT = None  # BassKernelResults of the most recent run (for profiling)


def kernel(**inputs):
    x = inputs["x"]
    last_pos = inputs["last_pos"]
    lp, per_core = _prep_inputs(
        x,
        last_pos,
        inputs["rope_cache"],
        inputs["wqkv"],
        inputs["o_proj_w"],
        inputs["cache_k"],
        inputs["cache_v"],
    )
    if lp not in _NC_CACHE:
        _NC_CACHE[lp] = _build_bass(lp)
    nc = _NC_CACHE[lp]
    res = run_bass_kernel_spmd(nc, per_core, core_ids=list(range(NCORES)))
    global LAST_RESULT
    LAST_RESULT = res
    results = res.results if hasattr(res, "results") else res
    out = np.zeros((B, H), np.float64)
    for c in range(NCORES):
        out += results[c]["out_p"].astype(np.float64)
    return out.astype(np.float32).reshape(B, 1, H)
